# revision 1
# baseline (speedup 1.0000x reference)
"""Causal self-attention (B=4, T=2048, C=2048, H=16, rope) on 8 trn2 cores.

Sharding: tensor-parallel over heads — 2 heads per core. Each core computes
q/k/v projections for its head slice from the full x, runs causal attention,
and produces a partial output projection y_c = attn_c @ wo[:, slice].T.
The host sums the 8 partial y tensors (row-parallel linear unshard).

Kernel layout (per core, "T" suffix = transposed so the contraction dim
sits on SBUF partitions):
  qT/kT [d=128, t]  <- wT (stationary) x xT (moving) matmuls + rope on DVE
  v     [t=128, d]  <- xT-tile (stationary) x wvT (moving) matmuls
  S^T   [j, i]      <- kT-tile (stationary) x qT-block (moving)
  P^T = exp(S^T * scale) with additive causal mask pre-exp
  attn^T [d, i]     <- v-tile (stationary) x P^T (moving), PSUM-accumulated
  rowsum broadcast  <- ones[128,128] (stationary) x rs-partial (moving)
  y[n, j]           <- attnT-tile (stationary) x woT (moving)
"""

import numpy as np

import concourse.bass as bass
import concourse.mybir as mybir
import concourse.tile as tile
from concourse.vector_clock import ScopedClock
from concourse.bass_utils import run_bass_kernel_spmd

# ---------------------------------------------------------------- tile patch
# The pinned walrus codegen accepts at most ONE sync-wait per hardware
# instruction; Tile attaches several. Split extras onto same-engine NOPs.

_MAX_WAITS = 1
_orig_add_instruction = tile.TileContext._add_instruction


def _split_add_instruction(self, inst):
    si = getattr(inst, "sync_info", None)
    if si is not None and len(si.on_wait) > _MAX_WAITS:
        waits = list(si.on_wait)
        extras, keep = waits[:-_MAX_WAITS], waits[-_MAX_WAITS:]
        inst.sync_info = mybir.SyncInfo(on_wait=keep, on_update=list(si.on_update))
        for i in range(0, len(extras), _MAX_WAITS):
            nop = mybir.InstNoOp(
                name=f"{inst.name}-ws{i}",
                sync_info=mybir.SyncInfo(on_wait=extras[i : i + _MAX_WAITS], on_update=[]),
                engine=inst.engine,
                bass_nofuse=True,
            )
            _orig_add_instruction(self, nop)
    _orig_add_instruction(self, inst)


def _patched_drain_and_barrier(self, tick_clock, wait_clock):
    nc = self.nc
    drain_inst = nc.sync.drain()
    wait_clock.add_sem_waits(drain_inst.ins, ScopedClock({None: tick_clock.global_clock}))
    si = drain_inst.ins.sync_info
    if si is not None and len(si.on_wait) > 1:
        waits = list(si.on_wait)
        drain_inst.ins.sync_info = mybir.SyncInfo(on_wait=waits[:1], on_update=list(si.on_update))
        for w in waits[1:]:
            extra = nc.sync.drain()
            extra.ins.sync_info = mybir.SyncInfo(on_wait=[w], on_update=[])
    nc.all_engine_barrier()
    assert self.sems is not None
    popped = nc._tile_sem_poison_stack.pop()
    assert popped is self._sem_poison
    nc.clear_and_free_semaphores(list(self.sems.allocated().values()))
    nc.all_engine_barrier()


tile.TileContext._add_instruction = _split_add_instruction
tile.TileContext._drain_and_barrier = _patched_drain_and_barrier

# ---------------------------------------------------------------- constants

B, T, C, H, D = 4, 2048, 2048, 16, 128
N_CORES = 8
HPC = H // N_CORES        # heads per core = 2
M = HPC * D               # per-core projection width = 256
BT = B * T
KT = C // 128             # 16 k-subtiles
SCALE = 1.0 / float(np.sqrt(D))
NEG = -30000.0            # pre-scale additive mask value; exp(scale*(s+NEG)) == 0

F32 = mybir.dt.float32
F32R = mybir.dt.float32r

# matmul dtype for the heavy stages (flip to F32 for a full-precision run)
DT_MM = F32R
ALU = mybir.AluOpType
AF = mybir.ActivationFunctionType


def build_kernel(dt_mm=DT_MM, do_attn=True, do_proj=True, nrep=1, y_bf16=False, dma_only=False):
    nc = bass.Bass("TRN2", target_bir_lowering=False, debug=False)

    xT = nc.dram_tensor("xT", [BT // 512, 128, KT, 512], dt_mm, kind="ExternalInput").ap()
    wqT = nc.dram_tensor("wqT", [C, M], dt_mm, kind="ExternalInput").ap()
    wkT = nc.dram_tensor("wkT", [C, M], dt_mm, kind="ExternalInput").ap()
    wvT = nc.dram_tensor("wvT", [C, M], dt_mm, kind="ExternalInput").ap()
    woT = nc.dram_tensor("woT", [M, C], dt_mm, kind="ExternalInput").ap()
    cosT = nc.dram_tensor("cosT", [D, T], F32, kind="ExternalInput").ap()
    sinT = nc.dram_tensor("sinT", [D, T], F32, kind="ExternalInput").ap()
    maskA = nc.dram_tensor("maskA", [128, 896], mybir.dt.bfloat16, kind="ExternalInput").ap()
    ones = nc.dram_tensor("ones", [128, 128], dt_mm, kind="ExternalInput").ap()
    dt_y = mybir.dt.bfloat16 if y_bf16 else F32
    y = nc.dram_tensor("y", [BT // 128, C // 512, 128, 512], dt_y, kind="ExternalOutput").ap()


    with tile.TileContext(nc) as tc:
        with (
            tc.tile_pool(name="const", bufs=1) as constp,
            tc.tile_pool(name="cs", bufs=2) as csp,
            tc.tile_pool(name="xpool", bufs=2) as xpool,
            tc.tile_pool(name="qpool", bufs=2) as qpool,
            tc.tile_pool(name="kvpool", bufs=1) as kvpool,
            tc.tile_pool(name="attnpool", bufs=2) as attnpool,
            tc.tile_pool(name="ptpool", bufs=3) as ptpool,
            tc.tile_pool(name="tmp", bufs=3) as tmpp,
            tc.tile_pool(name="ystg", bufs=2) as ystg,
            tc.tile_pool(name="ps_main", bufs=2, space="PSUM") as ps_main,
            tc.tile_pool(name="ps_misc", bufs=4, space="PSUM") as ps_misc,
            tc.tile_pool(name="ps_av", bufs=2, space="PSUM") as ps_av,
        ):
            # ---- resident constants
            wq_sb = constp.tile([128, KT, M], dt_mm, tag="wq")
            wk_sb = constp.tile([128, KT, M], dt_mm, tag="wk")
            wv_sb = constp.tile([128, KT, M], dt_mm, tag="wv")
            nc.sync.dma_start(wq_sb[:], wqT.rearrange("(ko p) m -> p ko m", p=128))
            nc.sync.dma_start(wk_sb[:], wkT.rearrange("(ko p) m -> p ko m", p=128))
            nc.sync.dma_start(wv_sb[:], wvT.rearrange("(ko p) m -> p ko m", p=128))
            wo_sb = constp.tile([128, HPC, C], dt_mm, tag="wo")
            nc.sync.dma_start(wo_sb[:], woT.rearrange("(mh p) j -> p mh j", p=128))
            mask_sb = constp.tile([128, 896], mybir.dt.bfloat16, tag="mask")
            nc.sync.dma_start(mask_sb[:], maskA[:])
            ones_sb = constp.tile([128, 128], dt_mm, tag="ones")
            nc.sync.dma_start(ones_sb[:], ones[:])

            wqk = [wq_sb, wq_sb, wk_sb, wk_sb]

            for _rep in range(nrep):
              for b in range(B):
                  # k/v for the whole sequence of this batch accumulate here
                  k_sb = kvpool.tile([D, HPC, T], dt_mm, tag="k")
                  v_sb = kvpool.tile([128, HPC, T // 128, D], dt_mm, tag="v")

                  for a in range(4):  # 512-token block (QKV -> attn -> proj)
                      t0 = a * 512
                      q_sb = qpool.tile([D, HPC, 512], dt_mm, tag="q")
                      attn_sb = attnpool.tile([D, HPC, 512], dt_mm, tag="attn")

                      # -------- phase A: qkv + rope for tokens [t0, t0+512)
                      x_t = xpool.tile([128, KT, 512], dt_mm, tag="x")
                      nc.sync.dma_start(x_t[:], xT[b * 4 + a])
                      if dma_only:
                          ydum = ystg.tile([128, 512], dt_y, tag="y")
                          nc.vector.tensor_copy(ydum[:], x_t[:, 0, :].bitcast(F32) if not y_bf16 else x_t[:, 0, :])
                          for nt in range(4):
                              for jb in range(4):
                                  rt = (b * T + a * 512 + nt * 128) // 128
                                  nc.sync.dma_start(y[rt, jb], ydum[:])
                          continue
                      cos_t = csp.tile([D, 512], F32, tag="cos")
                      sin_t = csp.tile([D, 512], F32, tag="sin")
                      nc.sync.dma_start(cos_t[:], cosT[:, t0 : t0 + 512])
                      nc.sync.dma_start(sin_t[:], sinT[:, t0 : t0 + 512])

                      for m in range(4):  # q0 q1 k0 k1
                          h = m % 2
                          ps = ps_main.tile([128, 512], F32, tag="mm", name="mm")
                          w_sb = wqk[m]
                          for kt in range(KT):
                              nc.tensor.matmul(
                                  ps[:],
                                  w_sb[:, kt, h * D : (h + 1) * D],
                                  x_t[:, kt, :],
                                  start=(kt == 0),
                                  stop=(kt == KT - 1),
                              )
                          rot = tmpp.tile([128, 512], F32, tag="tmp", name="rot")
                          t1 = tmpp.tile([128, 512], F32, tag="tmp", name="t1")
                          nc.vector.tensor_scalar_mul(rot[0:64, :], ps[64:128, :], -1.0)
                          nc.vector.tensor_copy(rot[64:128, :], ps[0:64, :])
                          nc.vector.tensor_tensor(t1[:], ps[:], cos_t[:], ALU.mult)
                          nc.vector.tensor_tensor(rot[:], rot[:], sin_t[:], ALU.mult)
                          dst = q_sb if m < 2 else k_sb
                          col = 0 if m < 2 else t0
                          nc.vector.tensor_tensor(
                              dst[:, h, col : col + 512], t1[:], rot[:], ALU.add
                          )

                      for nt in range(4):  # v in [t, d] layout directly
                          vp_full = ps_misc.tile([128, 512], F32, tag="misc", name="vp")
                          vp = vp_full[:, :M]
                          for kt in range(KT):
                              nc.tensor.matmul(
                                  vp,
                                  x_t[:, kt, nt * 128 : (nt + 1) * 128],
                                  wv_sb[:, kt, :],
                                  start=(kt == 0),
                                  stop=(kt == KT - 1),
                              )
                          jt = a * 4 + nt
                          for h in range(HPC):
                              nc.vector.tensor_copy(
                                  v_sb[:, h, jt, :], vp[:, h * D : (h + 1) * D]
                              )

                      if not do_attn:
                          continue

                      # -------- phase B: attention for i-block a, both heads
                      njt = 4 * a + 4
                      for h in range(HPC):
                          av = ps_av.tile([128, 512], F32, tag="av")
                          rsp = ps_misc.tile([128, 512], F32, tag="misc", name="rsp")
                          for jt in range(njt):
                              sp = ps_misc.tile([128, 512], F32, tag="misc")
                              nc.tensor.matmul(
                                  sp[:],
                                  k_sb[:, h, jt * 128 : (jt + 1) * 128],
                                  q_sb[:, h, :],
                                  start=True,
                                  stop=True,
                              )
                              if jt >= 4 * a:  # diagonal block: additive causal mask
                                  o = jt * 128 - a * 512
                                  nc.vector.tensor_tensor(
                                      sp[:], sp[:], mask_sb[:, 384 - o : 896 - o], ALU.add
                                  )
                              pt = ptpool.tile([128, 512], dt_mm, tag="pt")
                              nc.scalar.activation(pt[:], sp[:], AF.Exp, scale=SCALE)
                              nc.tensor.matmul(
                                  av[:],
                                  v_sb[:, h, jt, :],
                                  pt[:],
                                  start=(jt == 0),
                                  stop=(jt == njt - 1),
                              )
                              # rowsum (broadcast to all partitions) on the PE
                              nc.tensor.matmul(
                                  rsp[:],
                                  ones_sb[:],
                                  pt[:],
                                  start=(jt == 0),
                                  stop=(jt == njt - 1),
                              )
                          rec = tmpp.tile([128, 512], F32, tag="tmp")
                          nc.vector.reciprocal(rec[:], rsp[:])
                          nc.vector.tensor_tensor(attn_sb[:, h, :], av[:], rec[:], ALU.mult)

                      if not do_proj:
                          continue

                      # -------- phase C: output projection for this block
                      for nt in range(4):
                          for jb in range(4):
                              yp = ps_main.tile([128, 512], F32, tag="mm")
                              for mh in range(HPC):
                                  nc.tensor.matmul(
                                      yp[:],
                                      attn_sb[:, mh, nt * 128 : (nt + 1) * 128],
                                      wo_sb[:, mh, jb * 512 : (jb + 1) * 512],
                                      start=(mh == 0),
                                      stop=(mh == HPC - 1),
                                  )
                              yt = ystg.tile([128, 512], dt_y, tag="y")
                              nc.vector.tensor_copy(yt[:], yp[:])
                              rt = (b * T + a * 512 + nt * 128) // 128
                              nc.sync.dma_start(y[rt, jb], yt[:])
    return nc


_NC_CACHE = {}


def _get_nc(dt_mm=None, **kw):
    if dt_mm is None:
        dt_mm = DT_MM
    key = (str(dt_mm), tuple(sorted(kw.items())))
    if key not in _NC_CACHE:
        _NC_CACHE[key] = build_kernel(dt_mm, **kw)
    return _NC_CACHE[key]


def make_inputs(x, freqs_cos, freqs_sin, wq, wk, wv, wo):
    """Host-side shard prep: returns in_maps for the 8 cores."""
    x = np.asarray(x, dtype=np.float32)
    # blocked xT: [BT/512 blocks, 128 p, KT, 512 tokens], contiguous per block
    xT = np.ascontiguousarray(
        x.reshape(BT // 512, 512, KT, 128).transpose(0, 3, 2, 1)
    )
    cosT = np.ascontiguousarray(np.asarray(freqs_cos, np.float32).T)
    sinT = np.ascontiguousarray(np.asarray(freqs_sin, np.float32).T)
    p = np.arange(128)[:, None]
    g = np.arange(896)[None, :]
    # additive pre-scale mask: 0 where j<=i (valid), NEG where masked
    import ml_dtypes
    maskA = np.where(p <= g - 384, 0.0, NEG).astype(ml_dtypes.bfloat16)
    ones = np.ones((128, 128), np.float32)
    in_maps = []
    for c in range(N_CORES):
        sl = slice(c * M, (c + 1) * M)
        in_maps.append(
            {
                "xT": xT,
                "wqT": np.ascontiguousarray(np.asarray(wq, np.float32)[sl, :].T),
                "wkT": np.ascontiguousarray(np.asarray(wk, np.float32)[sl, :].T),
                "wvT": np.ascontiguousarray(np.asarray(wv, np.float32)[sl, :].T),
                "woT": np.ascontiguousarray(np.asarray(wo, np.float32)[:, sl].T),
                "cosT": cosT,
                "sinT": sinT,
                "maskA": maskA,
                "ones": ones,
            }
        )
    return in_maps


def kernel(x, freqs_cos, freqs_sin, wq, wk, wv, wo):
    nc = _get_nc()
    in_maps = make_inputs(x, freqs_cos, freqs_sin, wq, wk, wv, wo)
    res = run_bass_kernel_spmd(nc, in_maps, list(range(N_CORES)))
    out = np.zeros((BT // 128, C // 512, 128, 512), np.float64)
    for r in res.results:
        out += r["y"].astype(np.float64)
    out = out.astype(np.float32)
    # un-block: [BT/128, C/512, 128, 512] -> [BT, C]
    return out.transpose(0, 2, 1, 3).reshape(B, T, C)



# revision 5
# speedup vs baseline: 1.5823x; 1.5823x over previous
"""Causal self-attention (B=4, T=2048, C=2048, H=16, rope) on 8 trn2 cores.

Sharding: tensor-parallel over heads — 2 heads per core. Each core computes
q/k/v projections for its head slice from the full x, runs causal attention,
and produces a partial output projection y_c = attn_c @ wo[:, slice].T.
The host sums the 8 partial y tensors (row-parallel linear unshard).

v2 layout notes (vs the original baseline):
  - all matmul operands bf16 (fp32 PSUM accumulation) — FWL halves LDW cost
  - the two heads of a core are PAIRED into [128, 1024] tiles (2 PSUM banks)
    so every DVE/ACT instruction covers both heads (halves per-inst overhead)
  - causal trapezoid: the S/AV/rowsum matmuls for diagonal j-tiles only
    cover the valid i-columns (moving operand sliced, N = 512-128*s)
  - rope: 3 DVE ops per pair (sign folded into a host-prepped sinR tensor,
    half-swap done via cross-partition APs) + 1 add
  - PSUM->SBUF evacuations (v, y) run on ScalarE (fast PSUM port), freeing DVE
  - rowsum broadcast stays on the PE (ones-stationary matmul, PSUM-accumulated)
"""

import numpy as np

import concourse.bass as bass
import concourse.mybir as mybir
import concourse.tile as tile
from concourse.vector_clock import ScopedClock
from concourse.bass_utils import run_bass_kernel_spmd

# ---------------------------------------------------------------- tile patch
# The pinned walrus codegen accepts at most ONE sync-wait per hardware
# instruction; Tile attaches several. Split extras onto same-engine NOPs.

_MAX_WAITS = 1
_orig_add_instruction = tile.TileContext._add_instruction


def _split_add_instruction(self, inst):
    si = getattr(inst, "sync_info", None)
    if si is not None and len(si.on_wait) > _MAX_WAITS:
        waits = list(si.on_wait)
        extras, keep = waits[:-_MAX_WAITS], waits[-_MAX_WAITS:]
        inst.sync_info = mybir.SyncInfo(on_wait=keep, on_update=list(si.on_update))
        for i in range(0, len(extras), _MAX_WAITS):
            nop = mybir.InstNoOp(
                name=f"{inst.name}-ws{i}",
                sync_info=mybir.SyncInfo(on_wait=extras[i : i + _MAX_WAITS], on_update=[]),
                engine=inst.engine,
                bass_nofuse=True,
            )
            _orig_add_instruction(self, nop)
    _orig_add_instruction(self, inst)


def _patched_drain_and_barrier(self, tick_clock, wait_clock):
    nc = self.nc
    drain_inst = nc.sync.drain()
    wait_clock.add_sem_waits(drain_inst.ins, ScopedClock({None: tick_clock.global_clock}))
    si = drain_inst.ins.sync_info
    if si is not None and len(si.on_wait) > 1:
        waits = list(si.on_wait)
        drain_inst.ins.sync_info = mybir.SyncInfo(on_wait=waits[:1], on_update=list(si.on_update))
        for w in waits[1:]:
            extra = nc.sync.drain()
            extra.ins.sync_info = mybir.SyncInfo(on_wait=[w], on_update=[])
    nc.all_engine_barrier()
    assert self.sems is not None
    popped = nc._tile_sem_poison_stack.pop()
    assert popped is self._sem_poison
    nc.clear_and_free_semaphores(list(self.sems.allocated().values()))
    nc.all_engine_barrier()


tile.TileContext._add_instruction = _split_add_instruction
tile.TileContext._drain_and_barrier = _patched_drain_and_barrier

# ---------------------------------------------------------------- constants

B, T, C, H, D = 4, 2048, 2048, 16, 128
N_CORES = 8
HPC = H // N_CORES        # heads per core = 2
M = HPC * D               # per-core projection width = 256
BT = B * T
KT = C // 128             # 16 k-subtiles
SCALE = 1.0 / float(np.sqrt(D))
NEG = -30000.0            # pre-scale additive mask value; exp(scale*(s+NEG)) == 0

F32 = mybir.dt.float32
BF16 = mybir.dt.bfloat16

ALU = mybir.AluOpType
AF = mybir.ActivationFunctionType


def build_kernel(dt_mm=BF16, nrep=1):
    nc = bass.Bass("TRN2", target_bir_lowering=False, debug=False)

    xT = nc.dram_tensor("xT", [BT // 512, 128, KT, 512], dt_mm, kind="ExternalInput").ap()
    wqT = nc.dram_tensor("wqT", [C, M], dt_mm, kind="ExternalInput").ap()
    wkT = nc.dram_tensor("wkT", [C, M], dt_mm, kind="ExternalInput").ap()
    wvT = nc.dram_tensor("wvT", [C, M], dt_mm, kind="ExternalInput").ap()
    woT = nc.dram_tensor("woT", [M, C], dt_mm, kind="ExternalInput").ap()
    cosA = nc.dram_tensor("cosA", [D, 4, 1024], F32, kind="ExternalInput").ap()
    sinR = nc.dram_tensor("sinR", [D, 4, 1024], F32, kind="ExternalInput").ap()
    maskT = nc.dram_tensor("maskT", [128, 128], BF16, kind="ExternalInput").ap()
    ones = nc.dram_tensor("ones", [128, 128], dt_mm, kind="ExternalInput").ap()
    y = nc.dram_tensor("y", [BT // 128, 2, 128, 1024], BF16, kind="ExternalOutput").ap()

    with tile.TileContext(nc) as tc:
        with (
            tc.tile_pool(name="const", bufs=1) as constp,
            tc.tile_pool(name="xpool", bufs=2) as xpool,
            tc.tile_pool(name="qpool", bufs=2) as qpool,
            tc.tile_pool(name="kvpool", bufs=1) as kvpool,
            tc.tile_pool(name="attnpool", bufs=2) as attnpool,
            tc.tile_pool(name="ptpool", bufs=3) as ptpool,
            tc.tile_pool(name="tmp", bufs=3) as tmpp,
            tc.tile_pool(name="ystg", bufs=3) as ystg,
            tc.tile_pool(name="ps_pair", bufs=2, space="PSUM") as ps_pair,
            tc.tile_pool(name="ps_av", bufs=1, space="PSUM") as ps_av,
            tc.tile_pool(name="ps_rs", bufs=1, space="PSUM") as ps_rs,
        ):
            # ---- resident constants
            wq_sb = constp.tile([128, KT, M], dt_mm, tag="wq")
            wk_sb = constp.tile([128, KT, M], dt_mm, tag="wk")
            wv_sb = constp.tile([128, KT, M], dt_mm, tag="wv")
            nc.sync.dma_start(wq_sb[:], wqT.rearrange("(ko p) m -> p ko m", p=128))
            nc.sync.dma_start(wk_sb[:], wkT.rearrange("(ko p) m -> p ko m", p=128))
            nc.sync.dma_start(wv_sb[:], wvT.rearrange("(ko p) m -> p ko m", p=128))
            wo_sb = constp.tile([128, HPC, C], dt_mm, tag="wo")
            nc.sync.dma_start(wo_sb[:], woT.rearrange("(mh p) j -> p mh j", p=128))
            mask_sb = constp.tile([128, 128], BF16, tag="mask")
            nc.sync.dma_start(mask_sb[:], maskT[:])
            ones_sb = constp.tile([128, 128], dt_mm, tag="ones")
            nc.sync.dma_start(ones_sb[:], ones[:])
            cos_sb = constp.tile([D, 4, 1024], F32, tag="cos")
            sin_sb = constp.tile([D, 4, 1024], F32, tag="sin")
            nc.sync.dma_start(cos_sb[:], cosA[:])
            nc.sync.dma_start(sin_sb[:], sinR[:])

            for _rep in range(nrep):
              for b in range(B):
                  # k/v for the whole sequence of this batch accumulate here
                  k_sb = kvpool.tile([D, HPC, T], dt_mm, tag="k")
                  v_sb = kvpool.tile([128, (T // 128) * M], dt_mm, tag="v")

                  for a in range(4):  # 512-token block (QKV -> attn -> proj)
                      t0 = a * 512

                      # -------- phase A: qkv + rope for tokens [t0, t0+512)
                      x_t = xpool.tile([128, KT, 512], dt_mm, tag="x")
                      nc.sync.dma_start(x_t[:], xT[b * 4 + a])

                      q_sb = qpool.tile([D, HPC * 512], dt_mm, tag="q")
                      for which, w_sb in ((0, wq_sb), (1, wk_sb)):
                          ps = ps_pair.tile([128, 1024], F32, tag="pair", name="qk")
                          for h in range(HPC):
                              for kt in range(KT):
                                  nc.tensor.matmul(
                                      ps[:, h * 512 : h * 512 + 512],
                                      w_sb[:, kt, h * D : (h + 1) * D],
                                      x_t[:, kt, :],
                                      start=(kt == 0),
                                      stop=(kt == KT - 1),
                                  )
                          # rope on the [128, 1024] head-pair
                          t1 = tmpp.tile([128, 1024], F32, tag="t1")
                          rot = tmpp.tile([128, 1024], F32, tag="rot")
                          nc.vector.tensor_tensor(t1[:], ps[:], cos_sb[:, a, :], ALU.mult)
                          nc.vector.tensor_tensor(
                              rot[0:64, :], ps[64:128, :], sin_sb[0:64, a, :], ALU.mult
                          )
                          nc.vector.tensor_tensor(
                              rot[64:128, :], ps[0:64, :], sin_sb[64:128, a, :], ALU.mult
                          )
                          if which == 0:
                              nc.vector.tensor_tensor(q_sb[:], t1[:], rot[:], ALU.add)
                          else:
                              dst = k_sb[:, :, t0 : t0 + 512]
                              nc.vector.tensor_tensor(dst, t1[:], rot[:], ALU.add)

                      # v in [t, d] layout: all 4 nt-chunks in one [128,1024] psum
                      vp = ps_rs.tile([128, 1024], F32, tag="rs", name="vp")
                      for nt in range(4):
                          for kt in range(KT):
                              nc.tensor.matmul(
                                  vp[:, nt * 256 : (nt + 1) * 256],
                                  x_t[:, kt, nt * 128 : (nt + 1) * 128],
                                  wv_sb[:, kt, :],
                                  start=(kt == 0),
                                  stop=(kt == KT - 1),
                              )
                      for p2 in range(2):
                          c0 = (a * 4 + 2 * p2) * 256
                          nc.scalar.copy(
                              v_sb[:, c0 : c0 + 512],
                              vp[:, p2 * 512 : (p2 + 1) * 512],
                          )

                      # -------- phase B: attention for i-block a, both heads
                      njt = 4 * a + 4
                      av = ps_av.tile([128, 1024], F32, tag="av")
                      rs = ps_rs.tile([128, 1024], F32, tag="rs", name="rs")
                      for jt in range(njt):
                          s = max(0, jt - 4 * a)
                          off = 128 * s
                          n = 512 - off
                          sp = ps_pair.tile([128, 2, 512], F32, tag="pair", name="sp")
                          for h in range(HPC):
                              nc.tensor.matmul(
                                  sp[:, h, 0:n],
                                  k_sb[:, h, jt * 128 : (jt + 1) * 128],
                                  q_sb[:, h * 512 + off : (h + 1) * 512],
                                  start=True,
                                  stop=True,
                              )
                          if jt >= 4 * a:  # diagonal: additive causal mask
                              for h in range(HPC):
                                  nc.vector.tensor_tensor(
                                      sp[:, h, 0:128],
                                      sp[:, h, 0:128],
                                      mask_sb[:],
                                      ALU.add,
                                  )
                          pt = ptpool.tile([128, 2, 512], dt_mm, tag="pt")
                          nc.scalar.activation(
                              pt[:, :, 0:n],
                              sp[:, :, 0:n],
                              AF.Exp,
                              scale=SCALE,
                          )
                          for h in range(HPC):
                              nc.tensor.matmul(
                                  av[:, h * 512 + off : (h + 1) * 512],
                                  v_sb[:, jt * 256 + h * D : jt * 256 + (h + 1) * D],
                                  pt[:, h, 0:n],
                                  start=(jt == 0),
                                  stop=(jt == njt - 1),
                              )
                              nc.tensor.matmul(
                                  rs[:, h * 512 + off : (h + 1) * 512],
                                  ones_sb[:],
                                  pt[:, h, 0:n],
                                  start=(jt == 0),
                                  stop=(jt == njt - 1),
                              )
                      rec = tmpp.tile([128, 1024], F32, tag="t1", name="rec")
                      nc.vector.reciprocal(rec[:], rs[:])
                      attn_sb = attnpool.tile([128, HPC * 512], dt_mm, tag="attn")
                      nc.vector.tensor_tensor(attn_sb[:], av[:], rec[:], ALU.mult)

                      # -------- phase C: output projection for this block
                      for nt in range(4):
                          for jp in range(2):  # pair of 512-wide j blocks
                              yp = ps_pair.tile([128, 1024], F32, tag="pair", name="yp")
                              for jh in range(2):
                                  jb = jp * 2 + jh
                                  for mh in range(HPC):
                                      nc.tensor.matmul(
                                          yp[:, jh * 512 : (jh + 1) * 512],
                                          attn_sb[:, mh * 512 + nt * 128 : mh * 512 + (nt + 1) * 128],
                                          wo_sb[:, mh, jb * 512 : (jb + 1) * 512],
                                          start=(mh == 0),
                                          stop=(mh == HPC - 1),
                                      )
                              yt = ystg.tile([128, 1024], BF16, tag="y")
                              nc.scalar.copy(yt[:], yp[:])
                              rt = (b * T + a * 512 + nt * 128) // 128
                              nc.sync.dma_start(y[rt, jp], yt[:])
    return nc


_NC_CACHE = {}


def _get_nc(dt_mm=None, **kw):
    if dt_mm is None:
        dt_mm = BF16
    key = (str(dt_mm), tuple(sorted(kw.items())))
    if key not in _NC_CACHE:
        _NC_CACHE[key] = build_kernel(dt_mm, **kw)
    return _NC_CACHE[key]


def make_inputs(x, freqs_cos, freqs_sin, wq, wk, wv, wo):
    """Host-side shard prep: returns in_maps for the 8 cores."""
    import ml_dtypes

    bf16 = ml_dtypes.bfloat16
    x = np.asarray(x, dtype=np.float32)
    # blocked xT: [BT/512 blocks, 128 p, KT, 512 tokens], contiguous per block
    xT = np.ascontiguousarray(
        x.reshape(BT // 512, 512, KT, 128).transpose(0, 3, 2, 1)
    ).astype(bf16)
    cosT = np.asarray(freqs_cos, np.float32).T  # [D, T]
    sinT = np.asarray(freqs_sin, np.float32).T
    # cosA[d, a, h*512+i] = cosT[d, a*512+i]  (both head-halves identical)
    cosA = np.empty((D, 4, 1024), np.float32)
    sinR = np.empty((D, 4, 1024), np.float32)
    for a in range(4):
        blk_c = cosT[:, a * 512 : (a + 1) * 512]
        blk_s = sinT[:, a * 512 : (a + 1) * 512]
        cosA[:, a, 0:512] = blk_c
        cosA[:, a, 512:1024] = blk_c
        srow = np.concatenate([-blk_s[0:64], blk_s[64:128]], axis=0)
        sinR[:, a, 0:512] = srow
        sinR[:, a, 512:1024] = srow
    p = np.arange(128)[:, None]
    u = np.arange(128)[None, :]
    maskT = np.where(u >= p, 0.0, NEG).astype(bf16)
    ones = np.ones((128, 128), bf16)
    in_maps = []
    for c in range(N_CORES):
        sl = slice(c * M, (c + 1) * M)
        in_maps.append(
            {
                "xT": xT,
                "wqT": np.ascontiguousarray(np.asarray(wq, np.float32)[sl, :].T).astype(bf16),
                "wkT": np.ascontiguousarray(np.asarray(wk, np.float32)[sl, :].T).astype(bf16),
                "wvT": np.ascontiguousarray(np.asarray(wv, np.float32)[sl, :].T).astype(bf16),
                "woT": np.ascontiguousarray(np.asarray(wo, np.float32)[:, sl].T).astype(bf16),
                "cosA": cosA,
                "sinR": sinR,
                "maskT": maskT,
                "ones": ones,
            }
        )
    return in_maps


def kernel(x, freqs_cos, freqs_sin, wq, wk, wv, wo):
    nc = _get_nc()
    in_maps = make_inputs(x, freqs_cos, freqs_sin, wq, wk, wv, wo)
    res = run_bass_kernel_spmd(nc, in_maps, list(range(N_CORES)))
    out = np.zeros((BT // 128, 2, 128, 1024), np.float64)
    for r in res.results:
        out += r["y"].astype(np.float64)
    out = out.astype(np.float32)
    # un-block: [BT/128, 2, 128, 1024] -> [BT, C]
    return out.transpose(0, 2, 1, 3).reshape(B, T, C)


# revision 19
# speedup vs baseline: 1.5835x; 1.0007x over previous
"""Causal self-attention (B=4, T=2048, C=2048, H=16, rope) on 8 trn2 cores.

Sharding: tensor-parallel over heads — 2 heads per core. Each core computes
q/k/v projections for its head slice from the full x, runs causal attention,
and produces a partial output projection y_c = attn_c @ wo[:, slice].T.
The host sums the 8 partial y tensors (row-parallel linear unshard).

v3 layout notes:
  - all matmul operands bf16 (fp32 PSUM accumulation)
  - the two heads of a core are PAIRED into [128, 1024] tiles (2 PSUM banks)
    so every DVE/ACT instruction covers both heads
  - causal trapezoid: S/AV matmuls for diagonal j-tiles only cover the valid
    i-columns (moving operand sliced, N = 512-128*s)
  - rope: ACT evacuates the qk PSUM pair to bf16 SBUF, then 4 DVE ops run in
    2x mode (sign folded into host-prepped sinR; half-swap via partition APs)
  - softmax denominator: pt tiles are summed on DVE (bf16 2x) into ptsum,
    one ones-stationary matmul per (block, head-pair) broadcasts the rowsum
  - PSUM: qk/S/yp rotate through a 3-buf pair pool (6 banks) + av pool (2)
  - x blocks are DMA-prefetched one iteration ahead
"""

import numpy as np

import concourse.bass as bass
import concourse.mybir as mybir
import concourse.tile as tile
from concourse.vector_clock import ScopedClock
from concourse.bass_utils import run_bass_kernel_spmd

# ---------------------------------------------------------------- tile patch
# The pinned walrus codegen accepts at most ONE sync-wait per hardware
# instruction; Tile attaches several. Split extras onto same-engine NOPs.

_MAX_WAITS = 1
_orig_add_instruction = tile.TileContext._add_instruction


def _split_add_instruction(self, inst):
    si = getattr(inst, "sync_info", None)
    if si is not None and len(si.on_wait) > _MAX_WAITS:
        waits = list(si.on_wait)
        extras, keep = waits[:-_MAX_WAITS], waits[-_MAX_WAITS:]
        inst.sync_info = mybir.SyncInfo(on_wait=keep, on_update=list(si.on_update))
        for i in range(0, len(extras), _MAX_WAITS):
            nop = mybir.InstNoOp(
                name=f"{inst.name}-ws{i}",
                sync_info=mybir.SyncInfo(on_wait=extras[i : i + _MAX_WAITS], on_update=[]),
                engine=inst.engine,
                bass_nofuse=True,
            )
            _orig_add_instruction(self, nop)
    _orig_add_instruction(self, inst)


def _patched_drain_and_barrier(self, tick_clock, wait_clock):
    nc = self.nc
    drain_inst = nc.sync.drain()
    wait_clock.add_sem_waits(drain_inst.ins, ScopedClock({None: tick_clock.global_clock}))
    si = drain_inst.ins.sync_info
    if si is not None and len(si.on_wait) > 1:
        waits = list(si.on_wait)
        drain_inst.ins.sync_info = mybir.SyncInfo(on_wait=waits[:1], on_update=list(si.on_update))
        for w in waits[1:]:
            extra = nc.sync.drain()
            extra.ins.sync_info = mybir.SyncInfo(on_wait=[w], on_update=[])
    nc.all_engine_barrier()
    assert self.sems is not None
    popped = nc._tile_sem_poison_stack.pop()
    assert popped is self._sem_poison
    nc.clear_and_free_semaphores(list(self.sems.allocated().values()))
    nc.all_engine_barrier()


tile.TileContext._add_instruction = _split_add_instruction
tile.TileContext._drain_and_barrier = _patched_drain_and_barrier

# ---------------------------------------------------------------- constants

B, T, C, H, D = 4, 2048, 2048, 16, 128
N_CORES = 8
HPC = H // N_CORES        # heads per core = 2
M = HPC * D               # per-core projection width = 256
BT = B * T
KT = C // 128             # 16 k-subtiles
SCALE = 1.0 / float(np.sqrt(D))
NEG = -30000.0            # pre-scale additive mask value; exp(scale*(s+NEG)) == 0

F32 = mybir.dt.float32
BF16 = mybir.dt.bfloat16

ALU = mybir.AluOpType
AF = mybir.ActivationFunctionType


def build_kernel(dt_mm=BF16, nrep=1):
    nc = bass.Bass("TRN2", target_bir_lowering=False, debug=False)

    xT = nc.dram_tensor("xT", [BT // 512, 128, KT, 512], dt_mm, kind="ExternalInput").ap()
    wqT = nc.dram_tensor("wqT", [C, M], dt_mm, kind="ExternalInput").ap()
    wkT = nc.dram_tensor("wkT", [C, M], dt_mm, kind="ExternalInput").ap()
    wvT = nc.dram_tensor("wvT", [C, M], dt_mm, kind="ExternalInput").ap()
    woT = nc.dram_tensor("woT", [M, C], dt_mm, kind="ExternalInput").ap()
    cosA = nc.dram_tensor("cosA", [D, 4, 1024], BF16, kind="ExternalInput").ap()
    sinR = nc.dram_tensor("sinR", [D, 4, 1024], BF16, kind="ExternalInput").ap()
    maskT = nc.dram_tensor("maskT", [128, 256], BF16, kind="ExternalInput").ap()
    ones = nc.dram_tensor("ones", [128, 128], dt_mm, kind="ExternalInput").ap()
    y = nc.dram_tensor("y", [BT // 128, 2, 128, 1024], BF16, kind="ExternalOutput").ap()

    with tile.TileContext(nc) as tc:
        with (
            tc.tile_pool(name="const", bufs=1) as constp,
            tc.tile_pool(name="xpool", bufs=2) as xpool,
            tc.tile_pool(name="qpool", bufs=2) as qpool,
            tc.tile_pool(name="kvpool", bufs=1) as kvpool,
            tc.tile_pool(name="vpool", bufs=2) as vpool,
            tc.tile_pool(name="attnpool", bufs=2) as attnpool,
            tc.tile_pool(name="ptpool", bufs=3) as ptpool,
            tc.tile_pool(name="pspool", bufs=2) as pspool,
            tc.tile_pool(name="tmp", bufs=3) as tmpp,
            tc.tile_pool(name="ystg", bufs=3) as ystg,
            tc.tile_pool(name="ps_pair", bufs=2, space="PSUM") as ps_pair,
            tc.tile_pool(name="ps_av", bufs=1, space="PSUM") as ps_av,
            tc.tile_pool(name="ps_x", bufs=1, space="PSUM") as ps_x,
        ):
            # ---- resident constants (ordered by first use)
            wq_sb = constp.tile([128, KT, M], dt_mm, tag="wq")
            wk_sb = constp.tile([128, KT, M], dt_mm, tag="wk")
            wv_sb = constp.tile([128, KT, M], dt_mm, tag="wv")
            wq_r = wqT.rearrange("(ko p) m -> p ko m", p=128)
            x_first = xpool.tile([128, KT, 512], dt_mm, tag="x", name="xf")
            for kc in range(4):
                nc.sync.dma_start(wq_sb[:, kc * 4 : (kc + 1) * 4, :], wq_r[:, kc * 4 : (kc + 1) * 4, :])
                nc.sync.dma_start(
                    x_first[:, kc * 4 : (kc + 1) * 4, :], xT[0][:, kc * 4 : (kc + 1) * 4, :]
                )
            nc.sync.dma_start(wk_sb[:], wkT.rearrange("(ko p) m -> p ko m", p=128))
            nc.sync.dma_start(wv_sb[:], wvT.rearrange("(ko p) m -> p ko m", p=128))
            cos_sb = constp.tile([D, 4, 1024], BF16, tag="cos")
            sin_sb = constp.tile([D, 4, 1024], BF16, tag="sin")
            nc.sync.dma_start(cos_sb[:], cosA[:])
            nc.sync.dma_start(sin_sb[:], sinR[:])
            mask_sb = constp.tile([128, 2, 128], BF16, tag="mask")
            nc.sync.dma_start(mask_sb[:], maskT[:])
            ones_sb = constp.tile([128, 128], dt_mm, tag="ones")
            nc.sync.dma_start(ones_sb[:], ones[:])
            wo_sb = constp.tile([128, HPC, C], dt_mm, tag="wo")
            nc.sync.dma_start(wo_sb[:], woT.rearrange("(mh p) j -> p mh j", p=128))

            # deferred work from neighboring blocks, drained as PE filler
            # inside the attention loop: v-projection chains of the NEXT
            # block (ready immediately) and output-projection chains of the
            # PREVIOUS block (ready once its softmax tail completes)
            pending = []

            def make_proj(attn_sb, b, a):
                chains = []
                for nt in range(4):
                    for jp in range(2):
                        holder = {}
                        for jh in range(2):
                            def half(nt=nt, jp=jp, jh=jh, holder=holder,
                                     attn_sb=attn_sb, b=b, a=a):
                                if "yp" not in holder:
                                    holder["yp"] = ps_pair.tile(
                                        [128, 1024], F32, tag="pair", name="yp"
                                    )
                                yp = holder["yp"]
                                jb = jp * 2 + jh
                                for mh in range(HPC):
                                    nc.tensor.matmul(
                                        yp[:, jh * 512 : (jh + 1) * 512],
                                        attn_sb[:, mh * 512 + nt * 128 : mh * 512 + (nt + 1) * 128],
                                        wo_sb[:, mh, jb * 512 : (jb + 1) * 512],
                                        start=(mh == 0),
                                        stop=(mh == HPC - 1),
                                    )
                                if jh == 1:
                                    yt = ystg.tile([128, 1024], BF16, tag="y")
                                    nc.scalar.copy(yt[:], yp[:])
                                    rt = (b * T + a * 512 + nt * 128) // 128
                                    nc.sync.dma_start(y[rt, jp], yt[:])
                            chains.append(half)
                return chains

            def make_v(v_dst, x_src, a):
                """4 closures: nt-chains of the v projection into v_dst block a."""
                holder = {}
                pieces = []
                for nt in range(4):
                    for kh in range(2):
                        def piece(nt=nt, kh=kh, v_dst=v_dst, x_src=x_src, a=a):
                            if "vp" not in holder:
                                holder["vp"] = ps_x.tile([128, 1024], F32, tag="vp", name="vp")
                            vp = holder["vp"]
                            for kt in range(kh * 8, kh * 8 + 8):
                                nc.tensor.matmul(
                                    vp[:, nt * 256 : (nt + 1) * 256],
                                    x_src[:, kt, nt * 128 : (nt + 1) * 128],
                                    wv_sb[:, kt, :],
                                    start=(kt == 0),
                                    stop=(kt == KT - 1),
                                )
                            if nt % 2 == 1 and kh == 1:
                                p2 = nt // 2
                                c0 = (a * 4 + 2 * p2) * 256
                                nc.scalar.copy(
                                    v_dst[:, c0 : c0 + 512],
                                    vp[:, p2 * 512 : (p2 + 1) * 512],
                                )
                        pieces.append(piece)
                return pieces

            for _rep in range(nrep):
              if _rep == 0:
                  x_next = x_first
              else:
                  x_next = xpool.tile([128, KT, 512], dt_mm, tag="x", name="x0")
                  nc.sync.dma_start(x_next[:], xT[0])
              v_sb = vpool.tile([128, (T // 128) * M], dt_mm, tag="v", name="v0")
              for b in range(B):
                  # k for the whole sequence of this batch accumulates here
                  k_sb = kvpool.tile([D, HPC, T], dt_mm, tag="k")

                  for a in range(4):  # 512-token block (QKV -> attn -> proj)
                      # -------- phase A: qk + rope for tokens [t0, t0+512)
                      x_t = x_next
                      v_fill = []
                      if not (b == B - 1 and a == 3):
                          x_next = xpool.tile([128, KT, 512], dt_mm, tag="x", name="xn")
                          nc.sync.dma_start(x_next[:], xT[b * 4 + a + 1])
                          if a == 3:
                              v_sb_next = vpool.tile(
                                  [128, (T // 128) * M], dt_mm, tag="v", name="vn"
                              )
                          else:
                              v_sb_next = v_sb
                          v_fill = make_v(v_sb_next, x_next, (a + 1) % 4)

                      q_sb = qpool.tile([D, HPC * 512], dt_mm, tag="q")
                      for which, w_sb in ((0, wq_sb), (1, wk_sb)):
                          ps = ps_pair.tile([128, 1024], F32, tag="pair", name="qk")
                          for h in range(HPC):
                              for kt in range(KT):
                                  nc.tensor.matmul(
                                      ps[:, h * 512 : h * 512 + 512],
                                      w_sb[:, kt, h * D : (h + 1) * D],
                                      x_t[:, kt, :],
                                      start=(kt == 0),
                                      stop=(kt == KT - 1),
                                  )
                          # rope reads the PSUM pair directly (cross-partition
                          # SBUF-SBUF reads are rejected by the verifier)
                          t1 = tmpp.tile([128, 1024], F32, tag="t1")
                          rot = tmpp.tile([128, 1024], F32, tag="rot")
                          nc.vector.tensor_tensor(t1[:], ps[:], cos_sb[:, a, :], ALU.mult)
                          nc.vector.tensor_tensor(
                              rot[0:64, :], ps[64:128, :], sin_sb[0:64, a, :], ALU.mult
                          )
                          nc.vector.tensor_tensor(
                              rot[64:128, :], ps[0:64, :], sin_sb[64:128, a, :], ALU.mult
                          )
                          if which == 0:
                              nc.vector.tensor_tensor(q_sb[:], t1[:], rot[:], ALU.add)
                          else:
                              dst = k_sb[:, :, a * 512 : a * 512 + 512]
                              nc.vector.tensor_tensor(dst, t1[:], rot[:], ALU.add)

                      if b == 0 and a == 0:
                          # first block: no previous attention loop to hide in
                          for piece in make_v(v_sb, x_t, 0):
                              piece()

                      # -------- phase B: attention for i-block a, both heads
                      # (previous block's projection chains drain between the
                      # S and AV matmuls as PE filler for the exp latency)
                      njt = 4 * a + 4
                      pending = v_fill + pending
                      if a == 0:
                          # S0 here reads freshly-roped k; cover that latency
                          for _ in range(3):
                              if pending:
                                  pending.pop(0)()
                      av = ps_av.tile([128, 1024], F32, tag="av", name="av")
                      ptsum = pspool.tile([128, 2, 512], BF16, tag="ptsum")
                      for jt in range(njt):
                          s = max(0, jt - 4 * a)
                          off = 128 * s
                          n = 512 - off
                          sp = ps_pair.tile([128, 2, 512], F32, tag="pair", name="sp")
                          for h in range(HPC):
                              nc.tensor.matmul(
                                  sp[:, h, 0:n],
                                  k_sb[:, h, jt * 128 : (jt + 1) * 128],
                                  q_sb[:, h * 512 + off : (h + 1) * 512],
                                  start=True,
                                  stop=True,
                              )
                          if jt >= 4 * a:  # diagonal: additive causal mask
                              nc.vector.tensor_tensor(
                                  sp[:, :, 0:128],
                                  sp[:, :, 0:128],
                                  mask_sb[:],
                                  ALU.add,
                              )
                          pt = ptpool.tile([128, 2, 512], dt_mm, tag="pt")
                          nc.scalar.activation(
                              pt[:, :, 0:n],
                              sp[:, :, 0:n],
                              AF.Exp,
                              scale=SCALE,
                          )
                          for _ in range(2 if jt < 3 else 1):
                              if pending:
                                  pending.pop(0)()
                          for h in range(HPC):
                              nc.tensor.matmul(
                                  av[:, h * 512 + off : (h + 1) * 512],
                                  v_sb[:, jt * 256 + h * D : jt * 256 + (h + 1) * D],
                                  pt[:, h, 0:n],
                                  start=(jt == 0),
                                  stop=(jt == njt - 1),
                              )
                          # denominator accumulation on DVE (bf16 2x mode)
                          if jt == 0:
                              nc.vector.tensor_copy(ptsum[:], pt[:])
                          else:
                              nc.vector.tensor_tensor(
                                  ptsum[:, :, off:512],
                                  ptsum[:, :, off:512],
                                  pt[:, :, 0:n],
                                  ALU.add,
                              )
                      while pending:
                          pending.pop(0)()
                      # broadcast rowsum to all partitions via ones matmul
                      rs = ps_x.tile([128, 1024], F32, tag="vp", name="rs")
                      for h in range(HPC):
                          nc.tensor.matmul(
                              rs[:, h * 512 : (h + 1) * 512],
                              ones_sb[:],
                              ptsum[:, h, :],
                              start=True,
                              stop=True,
                          )
                      rec = tmpp.tile([128, 1024], F32, tag="rec")
                      nc.vector.reciprocal(rec[:], rs[:])
                      attn_sb = attnpool.tile([128, HPC * 512], dt_mm, tag="attn")
                      nc.vector.tensor_tensor(attn_sb[:], av[:], rec[:], ALU.mult)

                      # -------- phase C: defer projection into the next block
                      pending = make_proj(attn_sb, b, a)
                      if a == 3 and not (b == B - 1):
                          v_sb = v_sb_next
              while pending:
                  pending.pop(0)()
    return nc


_NC_CACHE = {}


def _get_nc(dt_mm=None, **kw):
    if dt_mm is None:
        dt_mm = BF16
    key = (str(dt_mm), tuple(sorted(kw.items())))
    if key not in _NC_CACHE:
        _NC_CACHE[key] = build_kernel(dt_mm, **kw)
    return _NC_CACHE[key]


def make_inputs(x, freqs_cos, freqs_sin, wq, wk, wv, wo):
    """Host-side shard prep: returns in_maps for the 8 cores."""
    import ml_dtypes

    bf16 = ml_dtypes.bfloat16
    x = np.asarray(x, dtype=np.float32)
    # blocked xT: [BT/512 blocks, 128 p, KT, 512 tokens], contiguous per block
    xT = np.ascontiguousarray(
        x.reshape(BT // 512, 512, KT, 128).transpose(0, 3, 2, 1)
    ).astype(bf16)
    cosT = np.asarray(freqs_cos, np.float32).T  # [D, T]
    sinT = np.asarray(freqs_sin, np.float32).T
    # cosA[d, a, h*512+i] = cosT[d, a*512+i]  (both head-halves identical)
    cosA = np.empty((D, 4, 1024), np.float32)
    sinR = np.empty((D, 4, 1024), np.float32)
    for a in range(4):
        blk_c = cosT[:, a * 512 : (a + 1) * 512]
        blk_s = sinT[:, a * 512 : (a + 1) * 512]
        cosA[:, a, 0:512] = blk_c
        cosA[:, a, 512:1024] = blk_c
        srow = np.concatenate([-blk_s[0:64], blk_s[64:128]], axis=0)
        sinR[:, a, 0:512] = srow
        sinR[:, a, 512:1024] = srow
    p = np.arange(128)[:, None]
    u = np.arange(128)[None, :]
    maskT = np.concatenate([np.where(u >= p, 0.0, NEG)] * 2, axis=1).astype(bf16)
    ones = np.ones((128, 128), bf16)
    in_maps = []
    for c in range(N_CORES):
        sl = slice(c * M, (c + 1) * M)
        in_maps.append(
            {
                "xT": xT,
                "wqT": np.ascontiguousarray(np.asarray(wq, np.float32)[sl, :].T).astype(bf16),
                "wkT": np.ascontiguousarray(np.asarray(wk, np.float32)[sl, :].T).astype(bf16),
                "wvT": np.ascontiguousarray(np.asarray(wv, np.float32)[sl, :].T).astype(bf16),
                "woT": np.ascontiguousarray(np.asarray(wo, np.float32)[:, sl].T).astype(bf16),
                "cosA": cosA.astype(bf16),
                "sinR": sinR.astype(bf16),
                "maskT": maskT,
                "ones": ones,
            }
        )
    return in_maps


def kernel(x, freqs_cos, freqs_sin, wq, wk, wv, wo):
    nc = _get_nc()
    in_maps = make_inputs(x, freqs_cos, freqs_sin, wq, wk, wv, wo)
    res = run_bass_kernel_spmd(nc, in_maps, list(range(N_CORES)))
    out = np.zeros((BT // 128, 2, 128, 1024), np.float64)
    for r in res.results:
        out += r["y"].astype(np.float64)
    out = out.astype(np.float32)
    # un-block: [BT/128, 2, 128, 1024] -> [BT, C]
    return out.transpose(0, 2, 1, 3).reshape(B, T, C)
